# revision 16
# baseline (speedup 1.0000x reference)
"""Trainium2 Bass kernel for LBP message-passing GNN (nn_NeuralNet_11252814315873).

Sharding: mention dim i across 8 cores (64 rows each). Per-core SPMD kernel:
  Phase A: matmuls for ER/eb/psi -> lbp_inputs block in DRAM
  Phase B: 10 LBP iterations; per-iteration AllToAll exchanges the mbar
           transpose block (column block) between cores.
Final ubar assembled on host from per-core mbar row blocks.

Device data layouts (per core p, rows I_p = [64p, 64p+64)):
  phase A:  phik/acc tiles [(il16*7+ei) 112-part, (j*7+ej) free]
  phase B:  partition q = jh*64+il  (jh = column half, il = my-row idx)
            lbp  [128, (ei, jl, ej)]  (12544 free)
            mbar_row [128, (jl, ej)]  (1792)
            mbar_col/s [128, (ei, jl)] (1792)
  a2a shard (src p -> dst q): [y(64), ej(7), x(64)]  (y = dst row, x = src row)
"""

import numpy as np

N = 512
C = 7
D = 300
KREL = 3
NCORES = 8
ROWS = N // NCORES        # 64
DPAD = 384                # padded contraction dim (3 chunks of 128)
LBP_LOOPS = 10
NEG = -30.0
SQRTD = float(np.sqrt(np.float32(D)))
LN_HALF = float(np.log(0.5))
JL = 256
FB = 1792                 # free block = 256*7
PHIK_F32R = True          # use float32r (fast fp32) for the big matmuls
DEBUG_TAPS = False        # extra outputs for debugging
TIMING_MODE = False       # drop collectives (for single-core TimelineSim)

_CACHE = {}


def _build():
    import concourse.bass as bass
    import concourse.bacc as bacc
    import concourse.tile as tile
    import concourse.mybir as mybir

    dt = mybir.dt
    f32 = dt.float32
    f32r = dt.float32r
    AF = mybir.ActivationFunctionType
    ALU = mybir.AluOpType
    AX = mybir.AxisListType

    nc = bacc.Bacc("TRN2", target_bir_lowering=False, debug=False,
                   num_devices=NCORES)

    def inp(name, shape):
        return nc.dram_tensor(name, shape, f32, kind="ExternalInput").ap()

    embT_my = inp("embT_my", [DPAD, 448])       # emb[I_p] flat (448,300).T pad
    fmcsT_myx = inp("fmcsT_myx", [DPAD, 448])   # fmcs[I_p] expanded over ei, .T
    fmcsAllT = inp("fmcsAllT", [DPAD, N])       # fmcs.T pad
    embAllT = inp("embAllT", [DPAD, 3584])      # aug: row300=1, row301=mask
    R_in = inp("R_in", [KREL, DPAD, DPAD])      # [k, d, e] pad
    D_in = inp("D_in", [KREL, DPAD, DPAD])      # [k, d, e] pad
    BT_in = inp("BT_in", [DPAD, DPAD])          # B.T pad  [e, d]
    Wsel_in = inp("Wsel", [128, 128])           # Trep selector
    ones_in = inp("ones_in", [1, 448])
    maskC_in = inp("maskC", [128, FB])          # maskf in [q,(jl,ej)] layout
    ident_in = inp("ident", [128, 128])

    out_mbar = nc.dram_tensor("out_mbar", [128, FB], f32,
                              kind="ExternalOutput").ap()
    out_psi = nc.dram_tensor("out_psi", [1, 448], f32,
                             kind="ExternalOutput").ap()
    if DEBUG_TAPS:
        out_lbp = nc.dram_tensor("out_lbp", [128, 7 * FB], f32,
                                 kind="ExternalOutput").ap()
        out_m1 = nc.dram_tensor("out_m1", [128, FB], f32,
                                kind="ExternalOutput").ap()
        out_mc2 = nc.dram_tensor("out_mc2", [128, FB], f32,
                                 kind="ExternalOutput").ap()

    def mm_cast(ap):
        return ap.bitcast(f32r) if PHIK_F32R else ap

    with tile.TileContext(nc) as tc:
        with (
            tc.tile_pool(name="dram", bufs=1, space="DRAM") as dram,
            tc.tile_pool(name="outer", bufs=1) as outer,
        ):
            lbp_sb = outer.tile([128, 7 * FB], f32, tag="lbp")
            a2a_in = dram.tile([NCORES, ROWS, 448], f32)
            a2a_out = dram.tile([NCORES, ROWS, 448], f32)

            # ======== Phase A: matmuls -> lbp_inputs in DRAM ========
            with tc.tile_pool(name="aper", bufs=1) as aper:
                # persistent within phase A
                ERT_sb = aper.tile([128, KREL, 3, 448],
                                   f32r if PHIK_F32R else f32, tag="ERT")
                AX_sb = aper.tile([112, 4, KREL, N], f32, tag="AX")
                psi_sb = aper.tile([1, 448], f32, tag="psi")

                # -------- A0: small matmuls --------
                with (
                    tc.tile_pool(name="a0", bufs=1) as a0,
                    tc.tile_pool(name="a0ps", bufs=2, space="PSUM") as a0ps,
                ):
                    embT_sb = a0.tile([128, 3, 448], f32r if PHIK_F32R else f32, tag="embT")
                    nc.gpsimd.dma_start(embT_sb[:], embT_my.rearrange(
                        "(c p) f -> p c f", p=128))
                    fmx_sb = a0.tile([128, 3, 448], f32r if PHIK_F32R else f32, tag="fmx")
                    nc.gpsimd.dma_start(fmx_sb[:], fmcsT_myx.rearrange(
                        "(c p) f -> p c f", p=128))
                    fa_sb = a0.tile([128, 3, N], f32r if PHIK_F32R else f32, tag="fa")
                    nc.gpsimd.dma_start(fa_sb[:], fmcsAllT.rearrange(
                        "(c p) f -> p c f", p=128))
                    RD_sb = a0.tile([128, KREL, 3, DPAD], f32r if PHIK_F32R else f32, tag="RD")
                    nc.gpsimd.dma_start(RD_sb[:], R_in.rearrange(
                        "k (c p) e -> p k c e", p=128))
                    BT_sb = a0.tile([128, 3, DPAD], f32r if PHIK_F32R else f32, tag="BT")
                    nc.gpsimd.dma_start(BT_sb[:], BT_in.rearrange(
                        "(c p) e -> p c e", p=128))

                    # ERT[e_chunkpart, (k, ec, (i,ei))] = ER_k^T
                    for k in range(KREL):
                        for et in range(3):
                            ps = a0ps.tile([128, 448], f32, tag="ps448")
                            for dc in range(3):
                                nc.tensor.matmul(
                                    ps[:],
                                    RD_sb[:, k, dc, et * 128:(et + 1) * 128],
                                    embT_sb[:, dc, :],
                                    start=(dc == 0), stop=(dc == 2))
                            nc.scalar.copy(ERT_sb[:, k, et, :], ps[:])

                    # vexpT[d, (i,ei)] = sum_e B[d,e] fmcs_exp[e,(i,ei)]
                    vexp_sb = a0.tile([128, 3, 448], f32, tag="vexp")
                    for dtile in range(3):
                        ps = a0ps.tile([128, 448], f32, tag="ps448")
                        for ec in range(3):
                            nc.tensor.matmul(
                                ps[:],
                                BT_sb[:, ec, dtile * 128:(dtile + 1) * 128],
                                fmx_sb[:, ec, :],
                                start=(ec == 0), stop=(ec == 2))
                        nc.scalar.copy(vexp_sb[:, dtile, :], ps[:])

                    # psiT[1,448] = sum_d embT * vexpT  (ones-matmul reduce)
                    prod_sb = a0.tile([128, 3, 448], f32, tag="prod")
                    nc.vector.tensor_tensor(prod_sb[:],
                                            embT_sb[:].bitcast(f32),
                                            vexp_sb[:], op=ALU.mult)
                    onescol_sb = a0.tile([128, 1], f32, tag="onescol")
                    nc.vector.memset(onescol_sb[:], 1.0)
                    psi_ps = a0ps.tile([1, 448], f32, tag="psips")
                    for dc in range(3):
                        nc.tensor.matmul(psi_ps[:], onescol_sb[:],
                                         prod_sb[:, dc, :],
                                         start=(dc == 0), stop=(dc == 2))
                    nc.scalar.copy(psi_sb[:], psi_ps[:])
                    nc.sync.dma_start(out_psi, psi_sb[:])
                    # aug rows of ERT (chunk 2, partitions 44=psi, 45=ones)
                    for k in range(KREL):
                        nc.gpsimd.dma_start(ERT_sb[44:45, k, 2, :], psi_sb[:])
                        nc.gpsimd.dma_start(ERT_sb[45:46, k, 2, :], ones_in)

                    # FDTx[e, (i,ei)] per k (expanded over ei)
                    nc.gpsimd.dma_start(RD_sb[:], D_in.rearrange(
                        "k (c p) e -> p k c e", p=128))
                    FDTx_sb = a0.tile([128, KREL, 3, 448],
                                      f32r if PHIK_F32R else f32, tag="FDTx")
                    for k in range(KREL):
                        for et in range(3):
                            ps = a0ps.tile([128, 448], f32, tag="ps448")
                            for dc in range(3):
                                nc.tensor.matmul(
                                    ps[:],
                                    RD_sb[:, k, dc, et * 128:(et + 1) * 128],
                                    fmx_sb[:, dc, :],
                                    start=(dc == 0), stop=(dc == 2))
                            nc.scalar.copy(FDTx_sb[:, k, et, :], ps[:])

                    # a (softmax over k) expanded: AX[(i,ei)-mtile, j]
                    for m in range(4):
                        for k in range(KREL):
                            ps = a0ps.tile([112, N], f32, tag="psE")
                            for ec in range(3):
                                nc.tensor.matmul(
                                    ps[:],
                                    FDTx_sb[:, k, ec,
                                            m * 112:(m + 1) * 112],
                                    fa_sb[:, ec, :],
                                    start=(ec == 0), stop=(ec == 2))
                            nc.scalar.activation(AX_sb[:, m, k, :], ps[:],
                                                 AF.Exp, scale=1.0 / SQRTD)
                    asum_sb = a0.tile([112, N], f32, tag="asum")
                    for m in range(4):
                        nc.vector.tensor_tensor(asum_sb[:], AX_sb[:, m, 0, :],
                                                AX_sb[:, m, 1, :], op=ALU.add)
                        nc.vector.tensor_tensor(asum_sb[:], asum_sb[:],
                                                AX_sb[:, m, 2, :], op=ALU.add)
                        nc.vector.reciprocal(asum_sb[:], asum_sb[:])
                        for k in range(KREL):
                            nc.vector.tensor_tensor(
                                AX_sb[:, m, k, :], AX_sb[:, m, k, :],
                                asum_sb[:], op=ALU.mult)

                # -------- A1: phik matmuls + a-combine --------
                with (
                    tc.tile_pool(name="a1", bufs=1) as a1,
                    tc.tile_pool(name="a1acc", bufs=5) as a1acc,
                    tc.tile_pool(name="a1ps", bufs=2, space="PSUM") as a1ps,
                ):
                    eaT_sb = a1.tile([128, 3, 3584],
                                     f32r if PHIK_F32R else f32, tag="eaT")
                    nc.gpsimd.dma_start(eaT_sb[:], embAllT.rearrange(
                        "(c p) f -> p c f", p=128))
                    for m in range(4):
                        for half in range(2):
                            acc = a1acc.tile([112, FB], f32, tag="acc")
                            tmp = a1acc.tile([112, FB], f32, tag="tmp")
                            for k in range(KREL):
                                ps = a1ps.tile([112, 2048], f32, tag="phik")
                                for ec in range(3):
                                    for b in range(4):
                                        nc.tensor.matmul(
                                            ps[:, b * 512:b * 512 + 448],
                                            ERT_sb[:, k, ec,
                                                   m * 112:(m + 1) * 112],
                                            eaT_sb[:, ec,
                                                   half * FB + b * 448:
                                                   half * FB + (b + 1) * 448],
                                            start=(ec == 0), stop=(ec == 2))
                                # a slice viewed per bank: (b, j64, ej)
                                aex = AX_sb[:, m, k,
                                            half * JL:(half + 1) * JL]
                                aexb = aex.rearrange(
                                    "p (b j) -> p b j", b=4)[:, :, :, None] \
                                    .broadcast_to([112, 4, 64, 7])
                                ps4 = ps[:].rearrange(
                                    "p (b c) -> p b c", b=4)[:, :, :448] \
                                    .rearrange("p b (j e) -> p b j e", e=7)
                                acc4 = acc[:].rearrange(
                                    "p (b j e) -> p b j e", b=4, e=7)
                                if k == 0:
                                    nc.vector.tensor_tensor(
                                        acc4, ps4, aexb, op=ALU.mult)
                                else:
                                    nc.vector.tensor_tensor(
                                        tmp[:].rearrange(
                                            "p (b j e) -> p b j e",
                                            b=4, e=7),
                                        ps4, aexb, op=ALU.mult)
                                    nc.vector.tensor_tensor(
                                        acc[:], acc[:], tmp[:], op=ALU.add)
                            # scatter SBUF->SBUF to lbp_sb [(jh il),(ei jl ej)]
                            for ei in range(7):
                                nc.sync.dma_start(
                                    lbp_sb[half * 64 + m * 16:
                                           half * 64 + m * 16 + 16,
                                           ei * FB:(ei + 1) * FB],
                                    acc[ei::7, :])

            # ================= Phase B: LBP iterations =================
            with (
                tc.tile_pool(name="lbp", bufs=1) as lp,
                tc.tile_pool(name="lbps", bufs=2, space="PSUM") as lps,
                tc.tile_pool(name="lbps3", bufs=3, space="PSUM") as lps3,
            ):
                if DEBUG_TAPS:
                    nc.sync.dma_start(out_lbp, lbp_sb[:])
                vals_sb = lp.tile([128, 7 * FB], f32, tag="vals")
                maskC_sb = lp.tile([128, FB], f32, tag="maskC")
                nc.sync.dma_start(maskC_sb[:], maskC_in[:])
                Wsel_sb = lp.tile([128, 128], f32, tag="Wsel")
                nc.sync.dma_start(Wsel_sb[:], Wsel_in[:])
                ident_sb = lp.tile([128, 128], f32, tag="ident")
                nc.sync.dma_start(ident_sb[:], ident_in[:])

                mbar_row = lp.tile([128, FB], f32, tag="mrow")
                mbar_col = lp.tile([128, FB], f32, tag="mcol")   # (ei, jl)
                s_sb = lp.tile([128, FB], f32, tag="s")          # (ei, jl)
                mval_sb = lp.tile([128, FB], f32, tag="mval")    # (jl, ej)
                expm_sb = lp.tile([128, FB], f32, tag="expm")
                hexp_sb = lp.tile([128, FB], f32, tag="hexp")
                cmax_sb = lp.tile([128, JL], f32, tag="cmax")
                den_sb = lp.tile([128, JL], f32, tag="den")
                Tpart_sb = lp.tile([128, 7], f32, tag="Tpart")
                stg_sb = lp.tile([ROWS, NCORES * 448], f32, tag="stg")
                lnhalf_sb = lp.tile([128, 1], f32, tag="lnhalf")
                nc.vector.memset(lnhalf_sb[:], LN_HALF)

                lbp3 = lbp_sb[:].rearrange("p (ei j e) -> p ei j e",
                                           ei=7, e=7)
                vals3 = vals_sb[:].rearrange("p (ei j e) -> p ei j e",
                                             ei=7, e=7)
                mval2 = mval_sb[:].rearrange("p (j e) -> p j e", e=7)
                expm2 = expm_sb[:].rearrange("p (j e) -> p j e", e=7)
                mrow_ej = mbar_row[:].rearrange("p (j e) -> p e j", e=7)

                for t in range(1, LBP_LOOPS + 1):
                    if t == 1:
                        src3 = lbp3
                    else:
                        # receive a2a -> mbar_col [(jh il), (ei, jl)]
                        for jh in range(2):
                            nc.sync.dma_start(
                                mbar_col[jh * 64:(jh + 1) * 64, :]
                                .rearrange("p (ei u x) -> p ei u x",
                                           ei=7, u=4),
                                a2a_out[jh * 4:(jh + 1) * 4]
                                .rearrange("u y (ei x) -> y ei u x", ei=7))
                        # T and s
                        nc.vector.tensor_reduce(
                            Tpart_sb[:],
                            mbar_col[:].rearrange("p (ei j) -> p ei j", ei=7),
                            axis=AX.X, op=ALU.add)
                        Trep_ps = lps.tile([128, 7], f32, tag="Trep")
                        nc.tensor.matmul(Trep_ps[:], Wsel_sb[:], Tpart_sb[:],
                                         start=True, stop=True)
                        nc.vector.tensor_tensor(
                            s_sb[:].rearrange("p (ei j) -> p ei j", ei=7),
                            Trep_ps[:][:, :, None].broadcast_to([128, 7, JL]),
                            mbar_col[:].rearrange("p (ei j) -> p ei j", ei=7),
                            op=ALU.subtract)
                        # vals = lbp + s (bcast over ej)
                        nc.vector.tensor_tensor(
                            vals3, lbp3,
                            s_sb[:].rearrange("p (ei j) -> p ei j", ei=7)
                            [:, :, :, None].broadcast_to([128, 7, JL, 7]),
                            op=ALU.add)
                        src3 = vals3

                    # max over ei (pairwise tree), into mval2 (uses expm as
                    # scratch)
                    nc.vector.tensor_tensor(mval2, src3[:, 0], src3[:, 1],
                                            op=ALU.max)
                    nc.vector.tensor_tensor(expm2, src3[:, 2], src3[:, 3],
                                            op=ALU.max)
                    nc.vector.tensor_tensor(mval2, mval2, expm2, op=ALU.max)
                    nc.vector.tensor_tensor(expm2, src3[:, 4], src3[:, 5],
                                            op=ALU.max)
                    nc.vector.tensor_tensor(expm2, expm2, src3[:, 6],
                                            op=ALU.max)
                    nc.vector.tensor_tensor(mval2, mval2, expm2, op=ALU.max)
                    # stabilize, exp, denom
                    nc.vector.tensor_reduce(cmax_sb[:], mval2, axis=AX.X,
                                            op=ALU.max)
                    nc.vector.tensor_tensor(
                        mval2, mval2,
                        cmax_sb[:][:, :, None].broadcast_to([128, JL, 7]),
                        op=ALU.subtract)
                    nc.scalar.activation(expm_sb[:], mval_sb[:], AF.Exp)
                    nc.vector.tensor_reduce(den_sb[:], expm2, axis=AX.X,
                                            op=ALU.add)
                    nc.vector.reciprocal(den_sb[:], den_sb[:])
                    if t > 1:
                        nc.scalar.activation(hexp_sb[:], mbar_row[:], AF.Exp,
                                             bias=lnhalf_sb[:])
                    # tail split by column half so PE transposes of half 0
                    # overlap DVE work on half 1
                    for h in range(2):
                        sl = slice(h * 896, (h + 1) * 896)
                        slq = slice(h * 128, (h + 1) * 128)
                        ex2 = expm_sb[:, sl].rearrange("p (j e) -> p j e",
                                                       e=7)
                        nc.vector.tensor_tensor(
                            ex2, ex2,
                            den_sb[:, slq][:, :, None].broadcast_to(
                                [128, 128, 7]),
                            op=ALU.mult)
                        if t == 1:
                            nc.vector.tensor_scalar(
                                mval_sb[:, sl], expm_sb[:, sl], 0.5, 0.5,
                                op0=ALU.mult, op1=ALU.add)
                        else:
                            nc.vector.scalar_tensor_tensor(
                                mval_sb[:, sl], expm_sb[:, sl], 0.5,
                                hexp_sb[:, sl],
                                op0=ALU.mult, op1=ALU.add)
                        nc.scalar.activation(mval_sb[:, sl], mval_sb[:, sl],
                                             AF.Ln)
                        nc.vector.tensor_tensor(mbar_row[:, sl],
                                                mval_sb[:, sl],
                                                maskC_sb[:, sl], op=ALU.mult)
                    if DEBUG_TAPS and t == 1:
                        nc.sync.dma_start(out_m1, mbar_row[:])
                    if DEBUG_TAPS and t == 2:
                        nc.sync.dma_start(out_mc2, mbar_col[:])

                    if t < LBP_LOOPS:
                        # transpose row block -> staged shards (y, ej, x)
                        for qp in range(4):
                            pst = lps3.tile([ROWS, 896], f32, tag="pst")
                            for ej in range(7):
                                nc.tensor.transpose(
                                    pst[:, ej * 128:(ej + 1) * 128],
                                    mrow_ej[:, ej,
                                            qp * 64:(qp + 1) * 64],
                                    ident_sb[:])
                            for jh in range(2):
                                q = jh * 4 + qp
                                nc.scalar.copy(
                                    stg_sb[:, q * 448:(q + 1) * 448]
                                    .rearrange("p (e x) -> p e x", e=7),
                                    pst[:].rearrange("p (e h x) -> p h e x",
                                                     e=7, h=2)[:, jh])
                        nc.sync.dma_start(
                            a2a_in[:].rearrange("q y f -> y q f"),
                            stg_sb[:].rearrange("p (q f) -> p q f",
                                                q=NCORES))
                        if not TIMING_MODE:
                            nc.gpsimd.collective_compute(
                                "AllToAll", mybir.AluOpType.bypass,
                                replica_groups=[list(range(NCORES))],
                                ins=[a2a_in.opt()], outs=[a2a_out.opt()])

                nc.sync.dma_start(out_mbar, mbar_row[:])

    nc.compile()
    return nc


def _prep_inputs(embeddings, fmcs, B, R, Dm, lengths):
    f = np.float32
    emb = np.asarray(embeddings, f)
    fmcs = np.asarray(fmcs, f)
    B = np.asarray(B, f)
    R = np.asarray(R, f)
    Dm = np.asarray(Dm, f)
    lengths = np.asarray(lengths)

    masks = (np.arange(C)[None, :] < (lengths[:, None] + 1))
    maskf = masks.astype(f)

    def padT(mat, cols):  # (rows<=DPAD, cols) zero-pad rows to DPAD
        out = np.zeros((DPAD, cols), f)
        out[:mat.shape[0], :] = mat
        return out

    embAllT = np.zeros((DPAD, 3584), f)
    embAllT[:D, :] = emb.reshape(3584, D).T
    embAllT[300, :] = 1.0
    embAllT[301, :] = np.where(masks.reshape(-1), 0.0, NEG).astype(f)

    fmcsAllT = padT(fmcs.T, N)

    Rpad = np.zeros((KREL, DPAD, DPAD), f)
    Rpad[:, :D, :D] = R
    Dpad = np.zeros((KREL, DPAD, DPAD), f)
    Dpad[:, :D, :D] = Dm
    BTpad = np.zeros((DPAD, DPAD), f)
    BTpad[:D, :D] = B.T

    Wsel = np.zeros((128, 128), f)
    il = np.arange(128)
    Wsel[il[:, None] % 64 == il[None, :] % 64] = 1.0
    ident = np.eye(128, dtype=f)
    onesrow = np.ones((1, 448), f)

    # maskC[jh*64+il, jl*7+ej] = maskf[jh*256+jl, ej]
    maskC = np.broadcast_to(
        maskf.reshape(2, JL, C)[:, None, :, :], (2, 64, JL, C))
    maskC = np.ascontiguousarray(maskC.reshape(128, FB), dtype=f)

    in_maps = []
    for p in range(NCORES):
        rows = slice(p * ROWS, (p + 1) * ROWS)
        embT_my = padT(emb[rows].reshape(448, D).T, 448)
        fmcsT_myx = padT(np.repeat(fmcs[rows], C, axis=0).T, 448)
        in_maps.append({
            "embT_my": embT_my, "fmcsT_myx": fmcsT_myx,
            "fmcsAllT": fmcsAllT,
            "embAllT": embAllT, "R_in": Rpad, "D_in": Dpad,
            "BT_in": BTpad, "Wsel": Wsel, "ones_in": onesrow,
            "maskC": maskC, "ident": ident,
        })
    return in_maps, maskf


def kernel(embeddings, fmcs, B, R, D, lengths):
    from concourse import bass_utils

    if "nc" not in _CACHE:
        _CACHE["nc"] = _build()
    nc = _CACHE["nc"]

    in_maps, maskf = _prep_inputs(embeddings, fmcs, B, R, D, lengths)
    res = bass_utils.run_bass_kernel_spmd(nc, in_maps,
                                          core_ids=list(range(NCORES)))
    _CACHE["last_results"] = res

    f = np.float32
    mbar = np.zeros((N, N, C), f)
    psi = np.zeros((N, C), f)
    for p in range(NCORES):
        blk = res.results[p]["out_mbar"].reshape(2, 64, JL, C)
        mbar[p * ROWS:(p + 1) * ROWS] = blk.transpose(1, 0, 2, 3).reshape(
            ROWS, N, C)
        psi[p * ROWS:(p + 1) * ROWS] = res.results[p]["out_psi"].reshape(
            ROWS, C)

    cnt = np.maximum(maskf.sum(-1), 1.0).astype(f)
    m2 = mbar * maskf[None, :, :]
    idx = np.arange(N)
    m2[idx, idx, :] = 0.0
    u = psi + m2.sum(0, dtype=f)
    u = u - ((u * maskf).sum(-1) / cnt)[:, None]
    ubar = np.exp(u) * maskf
    ssum = ubar.sum(-1, keepdims=True)
    ssum = np.where(ssum == 0, 1.0, ssum)
    return (ubar / ssum).astype(f)
